# revision 2
# baseline (speedup 1.0000x reference)
"""2-layer GAT (heads=4, concat=False, ELU between) on 8 Trainium2 cores.

Strategy (see DESIGN.md):
- Project-then-gather: dense phases compute XCAT[n] = [xh(n) fp16 (256) | als(n) f32 | pad]
  (768B rows) for every node; per-edge dma_gather fetches src rows (descriptor-bound,
  bytes are ~free). Same edge indices serve both layers.
- Core c owns dst nodes [c*6272, (c+1)*6272). Edges dst-sorted into 128-node dst-tiles;
  per tile a fixed number of 128-edge chunks (lo/hi split at node 32768 for int16 idx).
- Per chunk: one-hot selD (dst match) built on DVE; PE transpose -> selDT for the
  per-edge ald lookup (ald_e = selDT^T @ ald_tile); attention w = exp(lrelu(als+ald)-12)
  (constant shift cancels in softmax); G rows scaled by w (ACT per-partition scale);
  PSUM-accumulated scatter agg[d,0:256] += selD^T @ Gw with denominators in cols 256:260.
- Head-mean + ELU; h AllGathered between layers; output assembled on host.
"""
import sys
import os

sys.path.insert(0, '/opt/pypackages')
sys.path.insert(0, '/opt/trn_rl_repo')

import numpy as np

import concourse.bacc as bacc
import concourse.mybir as mybir
import concourse.tile as tile
from concourse.bass_utils import run_bass_kernel_spmd

F16 = mybir.dt.float16
F32 = mybir.dt.float32
I16 = mybir.dt.int16

NEG_SLOPE = 0.2
EXP_SHIFT = 0.0


class Cfg:
    def __init__(self, n, n_in, n_hid, n_out, heads, ncores, tiles_per_core,
                 split):
        self.N = n
        self.IN = n_in
        self.H = n_hid
        self.OUT = n_out
        self.HEADS = heads
        self.NCORES = ncores
        self.T = tiles_per_core              # dst-tiles per core
        self.NPC = tiles_per_core * 128      # nodes per core (padded)
        self.NPAD = ncores * self.NPC        # global padded node count
        self.SPLIT = split                   # int16 gather split boundary
        self.ROW = 384                       # fp16 elems per XCAT row (768B)
        self.XH = heads * n_hid              # 256 (=heads*OUT for layer 2)
        assert self.XH == 256 and self.ROW == 384


FULL = Cfg(50000, 128, 64, 64, 4, 8, 49, 32768)


def _wrap16(idx):
    """[n] int array -> [128, n//16] int16 dma_gather layout, replicated x8."""
    n = len(idx)
    assert n % 16 == 0
    base = np.asarray(idx, dtype=np.int16).reshape(n // 16, 16).T  # [16, n/16]
    return np.tile(base, (8, 1))


def host_prep(cfg, edge_index):
    """Build per-core gather indices / dstloc arrays. Returns dict."""
    src = np.asarray(edge_index[0], dtype=np.int64)
    dst = np.asarray(edge_index[1], dtype=np.int64)
    loops = np.arange(cfg.N, dtype=np.int64)
    src = np.concatenate([src, loops])
    dst = np.concatenate([dst, loops])

    core_of = dst // cfg.NPC
    tile_of = (dst % cfg.NPC) // 128

    # per (core, tile): lo/hi edge lists sorted by src
    lists = [[None] * cfg.T for _ in range(cfg.NCORES)]
    c_lo_max = c_hi_max = 1
    order = np.lexsort((src, tile_of, core_of))
    src_s, dst_s = src[order], dst[order]
    core_s, tile_s = core_of[order], tile_of[order]
    # boundaries
    key = core_s * cfg.T + tile_s
    starts = np.searchsorted(key, np.arange(cfg.NCORES * cfg.T), side='left')
    ends = np.searchsorted(key, np.arange(cfg.NCORES * cfg.T), side='right')
    for c in range(cfg.NCORES):
        for t in range(cfg.T):
            k = c * cfg.T + t
            s, e = starts[k], ends[k]
            es, ed = src_s[s:e], dst_s[s:e]
            lo = es < cfg.SPLIT
            lists[c][t] = (es[lo], ed[lo], es[~lo], ed[~lo])
            c_lo_max = max(c_lo_max, (len(es[lo]) + 127) // 128)
            c_hi_max = max(c_hi_max, (len(es[~lo]) + 127) // 128)
    C_lo, C_hi = c_lo_max, c_hi_max
    C = C_lo + C_hi

    gidx = np.zeros((cfg.NCORES, cfg.T, 128, C * 8), dtype=np.int16)
    dstloc = np.full((cfg.NCORES, cfg.T, 128, C), -1.0, dtype=np.float32)
    for c in range(cfg.NCORES):
        for t in range(cfg.T):
            base = (c * cfg.T + t) * 128
            es_lo, ed_lo, es_hi, ed_hi = lists[c][t]
            ilo = np.zeros(C_lo * 128, dtype=np.int64)
            ilo[:len(es_lo)] = es_lo
            ihi = np.zeros(C_hi * 128, dtype=np.int64)
            ihi[:len(es_hi)] = es_hi - cfg.SPLIT
            gidx[c, t, :, :C_lo * 8] = _wrap16(ilo)
            gidx[c, t, :, C_lo * 8:] = _wrap16(ihi)
            dl = np.full((C * 128,), -1.0, dtype=np.float32)
            dl[:len(ed_lo)] = (ed_lo - base).astype(np.float32)
            dl[C_lo * 128:C_lo * 128 + len(ed_hi)] = \
                (ed_hi - base).astype(np.float32)
            dstloc[c, t] = dl.reshape(C, 128).T
    # ald group-gather indices: groups of 16 nodes; per-core 392 -> pad 512
    gpc = cfg.NPC // 16  # groups per core
    aldg = np.zeros((cfg.NCORES, 128, (gpc + 127) // 128 * 8), dtype=np.int16)
    n_ald = ((gpc + 127) // 128) * 128
    for c in range(cfg.NCORES):
        g = np.zeros(n_ald, dtype=np.int64)
        g[:gpc] = c * gpc + np.arange(gpc)
        aldg[c] = _wrap16(g)
    return dict(C_lo=C_lo, C_hi=C_hi, C=C, gidx=gidx, dstloc=dstloc,
                aldg=aldg, n_ald=n_ald)


def _weights_cat(W, a_src, a_dst, heads, ch):
    """[Fin, heads*ch] + [heads, ch]x2 -> fp16 [Fin, heads*ch + 8]."""
    fin = W.shape[0]
    ws = np.einsum('fhc,hc->fh', W.reshape(fin, heads, ch), a_src)
    wd = np.einsum('fhc,hc->fh', W.reshape(fin, heads, ch), a_dst)
    out = np.zeros((fin, heads * ch + 8), dtype=np.float16)
    out[:, :heads * ch] = W.astype(np.float16)
    out[:, heads * ch:heads * ch + heads] = ws.astype(np.float16)
    out[:, heads * ch + heads:heads * ch + 2 * heads] = wd.astype(np.float16)
    return out


def build_kernel(cfg, C_lo, C_hi, n_ald):
    C = C_lo + C_hi
    nc = bacc.Bacc("TRN2", target_bir_lowering=False, debug=False,
                   num_devices=cfg.NCORES, num_swdge_queues=4)
    NP1 = ((cfg.N + 127) // 128) * 128        # XCAT1 rows (50048)
    T_G1 = NP1 // 128                          # global tiles layer 1 (391)
    NP2 = cfg.NPAD                             # XCAT2 rows (50176)
    T_G2 = NP2 // 128                          # 392

    x_in = nc.dram_tensor("x", [cfg.N, cfg.IN], F32, kind="ExternalInput")
    wa1 = nc.dram_tensor("wa1", [cfg.IN, 264], F16, kind="ExternalInput")
    wa2 = nc.dram_tensor("wa2", [cfg.H, 264], F16, kind="ExternalInput")
    mconst = nc.dram_tensor("mconst", [128, 128], F32, kind="ExternalInput")
    ident = nc.dram_tensor("ident", [128, 128], F32, kind="ExternalInput")
    gidx_d = nc.dram_tensor("gidx", [cfg.T, 128, C * 8], I16,
                            kind="ExternalInput")
    dstloc_d = nc.dram_tensor("dstloc", [cfg.T, 128, C], F32,
                              kind="ExternalInput")
    aldg_d = nc.dram_tensor("aldg", [128, n_ald // 16], I16,
                            kind="ExternalInput")
    out_d = nc.dram_tensor("out_slice", [cfg.NPC, cfg.OUT], F32,
                           kind="ExternalOutput")

    with tile.TileContext(nc) as tc:
        with tc.tile_pool(name="dram", bufs=1, space="DRAM") as dpool, \
             tc.tile_pool(name="const", bufs=1) as cpool, \
             tc.tile_pool(name="work", bufs=2) as pool, \
             tc.tile_pool(name="gpool", bufs=2) as gpool, \
             tc.tile_pool(name="gw", bufs=4) as gwpool, \
             tc.tile_pool(name="seld", bufs=C + 2) as sdpool, \
             tc.tile_pool(name="psum", bufs=2, space="PSUM") as psum, \
             tc.tile_pool(name="psA", bufs=2, space="PSUM") as psA, \
             tc.tile_pool(name="psB", bufs=2, space="PSUM") as psB:

            xs16 = dpool.tile([NP1, cfg.IN], F16, name="xs16", uniquify=False)
            xcat1 = dpool.tile([NP1, cfg.ROW], F16, name="xcat1", uniquify=False)
            aldf1 = dpool.tile([NP2, 4], F32, name="aldf1", uniquify=False)
            aldl1 = dpool.tile([n_ald * 16, 4], F32, name="aldl1", uniquify=False)
            h_loc = dpool.tile([cfg.NPC, 128], F32, name="h_loc", uniquify=False)
            h_full = dpool.tile([NP2, 128], F32, name="h_full", uniquify=False,
                                addr_space="Shared")
            h16 = dpool.tile([NP2, 128], F16, name="h16", uniquify=False)
            xcat2 = dpool.tile([NP2, cfg.ROW], F16, name="xcat2", uniquify=False)
            aldf2 = dpool.tile([NP2, 4], F32, name="aldf2", uniquify=False)
            aldl2 = dpool.tile([n_ald * 16, 4], F32, name="aldl2", uniquify=False)

            mconst_sb = cpool.tile([128, 128], F32)
            nc.sync.dma_start(out=mconst_sb[:], in_=mconst[:, :])
            ident_sb = cpool.tile([128, 128], F32)
            nc.sync.dma_start(out=ident_sb[:], in_=ident[:, :])
            wa1_sb = cpool.tile([cfg.IN, 264], F16)
            nc.sync.dma_start(out=wa1_sb[:], in_=wa1[:, :])
            wa2_sb = cpool.tile([cfg.H, 264], F16)
            nc.sync.dma_start(out=wa2_sb[:], in_=wa2[:, :])
            aldg_sb = cpool.tile([128, n_ald // 16], I16)
            nc.sync.dma_start(out=aldg_sb[:], in_=aldg_d[:, :])
            zero_sb = cpool.tile([128, 128], F16)
            nc.gpsimd.memset(zero_sb[:], 0)
            zero_f32 = cpool.tile([128, 8], F32)
            nc.gpsimd.memset(zero_f32[:], 0)
            zero64 = cpool.tile([128, 64], F32)
            nc.gpsimd.memset(zero64[:], 0)

            # ---- stage x -> fp16, zero pad rows ----
            nc.gpsimd.dma_start(
                out=xs16[0:cfg.N, :].flatten(),
                in_=x_in[:, :].flatten())
            if NP1 > cfg.N:
                npad = NP1 - cfg.N
                nc.sync.dma_start(out=xs16[cfg.N:NP1, :],
                                  in_=zero_sb[0:npad, 0:cfg.IN])
            # zero ALDF1 rows beyond layer-1 global tiles
            if NP2 > NP1:
                nc.sync.dma_start(out=aldf1[NP1:NP2, :],
                                  in_=zero_f32[0:NP2 - NP1, 0:4])

            def dense_phase(src16, n_rows, fin, wa_sb, xcat, aldf):
                """src16 [n_rows, 128-col fp16 staging] @ wa -> xcat + aldf."""
                BT = 8  # subtiles per batch
                B = BT * 128
                nb = 0
                bi = 0
                while nb < n_rows:
                    bsz = min(B, n_rows - nb)
                    st = bsz // 128
                    xT = pool.tile([128, B], F16, name=f"xT{id(xcat)}_{bi}",
                                   tag="xT")
                    nc.sync.dma_start(out=xT[:, 0:bsz],
                                      in_=src16[nb:nb + bsz, :],
                                      transpose=True)
                    xc = pool.tile([128, BT, 264], F16,
                                   name=f"xc{id(xcat)}_{bi}", tag="xc")
                    xcf = xc[:].bitcast(F32)  # [128, BT, 132]
                    arow = pool.tile([128, BT, 4], F32,
                                     name=f"ar{id(xcat)}_{bi}", tag="ar")
                    for s in range(st):
                        ps = psA.tile([128, 264], F32, name=f"dps{bi}_{s}",
                                      tag="dps")
                        nc.tensor.matmul(
                            ps[:], xT[0:fin, s * 128:(s + 1) * 128],
                            wa_sb[:], start=True, stop=True)
                        nc.scalar.activation(
                            xc[:, s, 0:256], ps[:, 0:256],
                            mybir.ActivationFunctionType.Copy)
                        nc.vector.tensor_copy(xcf[:, s, 128:132],
                                              ps[:, 256:260])
                        nc.vector.tensor_copy(arow[:, s, :], ps[:, 260:264])
                    nc.sync.dma_start(
                        out=xcat[nb:nb + bsz, 0:264].rearrange(
                            "(s p) d -> p s d", p=128),
                        in_=xc[:, 0:st, :])
                    nc.sync.dma_start(
                        out=aldf[nb:nb + bsz, :].rearrange(
                            "(s p) d -> p s d", p=128),
                        in_=arow[:, 0:st, :])
                    nb += bsz
                    bi += 1

            def ald_gather(aldf, aldl):
                asb = pool.tile([128, n_ald // 128, 64], F32, tag="asb")
                nc.gpsimd.dma_gather(
                    asb[:],
                    aldf[:, :].rearrange("(g k) d -> g (k d)", k=16),
                    aldg_sb[:], n_ald, n_ald, 64, single_packet=False)
                nc.sync.dma_start(
                    out=aldl[:, :].rearrange("(c p j) d -> p c (j d)",
                                             p=128, j=16),
                    in_=asb[:])

            def edge_sweep(xcat, n_rows, aldl, layer):
                for t in range(cfg.T):
                    q = t % 4
                    sfx = f"_{layer}_{t}"
                    idx_t = pool.tile([128, C * 8], I16, name="ix" + sfx,
                                      tag="ix")
                    nc.sync.dma_start(out=idx_t[:], in_=gidx_d[t, :, :])
                    dst_t = pool.tile([128, C], F32, name="dl" + sfx,
                                      tag="dl")
                    nc.sync.dma_start(out=dst_t[:], in_=dstloc_d[t, :, :])
                    ald_t = pool.tile([128, 4], F32, name="at" + sfx,
                                      tag="at")
                    nc.sync.dma_start(out=ald_t[:],
                                      in_=aldl[t * 128:(t + 1) * 128, :])
                    G = gpool.tile([128, C, cfg.ROW], F16, name="G" + sfx,
                                   tag="G")
                    nc.gpsimd.dma_gather(
                        G[:, 0:C_lo, :], xcat[0:cfg.SPLIT, :],
                        idx_t[:, 0:C_lo * 8], C_lo * 128, C_lo * 128,
                        cfg.ROW, single_packet=False, queue_num=q)
                    nc.gpsimd.dma_gather(
                        G[:, C_lo:C, :], xcat[cfg.SPLIT:n_rows, :],
                        idx_t[:, C_lo * 8:], C_hi * 128, C_hi * 128,
                        cfg.ROW, single_packet=False, queue_num=q)
                    Gf = G[:].bitcast(F32)  # [128, C, 192]

                    ald_ps = psB.tile([128, C * 4], F32, name="alp" + sfx,
                                      tag="alp")
                    sel = []
                    for c in range(C):
                        sd = sdpool.tile([128, 128], F32,
                                         name=f"sd{sfx}_{c}", tag="sd")
                        nc.vector.tensor_scalar(
                            sd[:], mconst_sb[:], dst_t[:, c:c + 1], None,
                            mybir.AluOpType.is_equal)
                        sel.append(sd)
                        trp = psum.tile([128, 128], F32,
                                        name=f"tr{sfx}_{c}", tag="tr")
                        nc.tensor.transpose(trp[:], sd[:], ident_sb[:])
                        sdt = pool.tile([128, 128], F32,
                                        name=f"st{sfx}_{c}", tag="st")
                        nc.vector.tensor_copy(sdt[:], trp[:])
                        nc.tensor.matmul(
                            ald_ps[:, c * 4:(c + 1) * 4], sdt[:], ald_t[:],
                            start=True, stop=True)

                    alde = pool.tile([128, C * 4], F32, name="ae" + sfx,
                                     tag="ae")
                    nc.vector.tensor_copy(alde[:], ald_ps[:])
                    alpha = pool.tile([128, C, 4], F32, name="alf" + sfx,
                                      tag="alf")
                    nc.vector.tensor_tensor(
                        out=alpha[:], in0=Gf[:, :, 128:132],
                        in1=alde[:].rearrange("p (c f) -> p c f", f=4),
                        op=mybir.AluOpType.add)
                    # lrelu = max(z, 0.2z); then w = exp(lrelu - 12)
                    alr = pool.tile([128, C, 4], F32, name="alr" + sfx,
                                    tag="alr")
                    nc.vector.tensor_scalar(
                        alr[:], alpha[:], NEG_SLOPE, None,
                        mybir.AluOpType.mult)
                    nc.vector.tensor_tensor(out=alr[:], in0=alr[:],
                                            in1=alpha[:],
                                            op=mybir.AluOpType.max)
                    w32 = pool.tile([128, C * 4], F32, name="w" + sfx,
                                    tag="w")
                    nc.scalar.activation(
                        w32[:].rearrange("p (c f) -> p c f", f=4), alr[:],
                        mybir.ActivationFunctionType.Exp)

                    agg = psB.tile([128, 260], F32, name="agg" + sfx,
                                   tag="agg")
                    for c in range(C):
                        gw = gwpool.tile([128, 264], F32,
                                         name=f"gw{sfx}_{c}", tag="gw")
                        for h in range(4):
                            nc.scalar.activation(
                                gw[:, h * 64:(h + 1) * 64],
                                G[:, c, h * 64:(h + 1) * 64],
                                mybir.ActivationFunctionType.Copy,
                                scale=w32[:, c * 4 + h:c * 4 + h + 1])
                        nc.vector.tensor_copy(gw[:, 256:260],
                                              w32[:, c * 4:(c + 1) * 4])
                        nc.tensor.matmul(
                            agg[:, 0:260], sel[c][:], gw[:, 0:260],
                            start=(c == 0), stop=(c == C - 1),
                            skip_group_check=True)

                    den = pool.tile([128, 4], F32, name="dn" + sfx, tag="dn")
                    nc.vector.tensor_scalar(den[:], agg[:, 256:260], 1e-16,
                                            None, mybir.AluOpType.max)
                    rec = pool.tile([128, 4], F32, name="rc" + sfx, tag="rc")
                    nc.vector.reciprocal(rec[:], den[:])
                    nc.vector.tensor_scalar(rec[:], rec[:], 0.25, None,
                                            mybir.AluOpType.mult)
                    tmp = pool.tile([128, 4, 64], F32, name="tm" + sfx,
                                    tag="tm")
                    for h in range(4):
                        nc.scalar.activation(
                            tmp[:, h, :], agg[:, h * 64:(h + 1) * 64],
                            mybir.ActivationFunctionType.Copy,
                            scale=rec[:, h:h + 1])
                    s0 = pool.tile([128, 128], F32, name="s0" + sfx,
                                   tag="s0")
                    nc.vector.tensor_copy(s0[:, 64:128], zero64[:])
                    nc.vector.tensor_tensor(out=s0[:, 0:64], in0=tmp[:, 0, :],
                                            in1=tmp[:, 1, :],
                                            op=mybir.AluOpType.add)
                    s1 = pool.tile([128, 64], F32, name="s1" + sfx, tag="s1")
                    nc.vector.tensor_tensor(out=s1[:], in0=tmp[:, 2, :],
                                            in1=tmp[:, 3, :],
                                            op=mybir.AluOpType.add)
                    nc.vector.tensor_tensor(out=s0[:, 0:64],
                                            in0=s0[:, 0:64], in1=s1[:],
                                            op=mybir.AluOpType.add)
                    if layer == 1:
                        # ELU(s) = max(s,0) + exp(min(s,0)) - 1
                        ng = pool.tile([128, 64], F32, name="ng" + sfx,
                                       tag="ng")
                        nc.vector.tensor_scalar(ng[:], s0[:, 0:64], 0.0,
                                                None, mybir.AluOpType.min)
                        ex = pool.tile([128, 64], F32, name="ex" + sfx,
                                       tag="ex")
                        nc.scalar.activation(
                            ex[:], ng[:], mybir.ActivationFunctionType.Exp)
                        nc.vector.tensor_scalar(s0[:, 0:64], s0[:, 0:64],
                                                0.0, None,
                                                mybir.AluOpType.max)
                        nc.vector.tensor_tensor(out=s0[:, 0:64],
                                                in0=s0[:, 0:64], in1=ex[:],
                                                op=mybir.AluOpType.add)
                        nc.vector.tensor_scalar(s0[:, 0:64], s0[:, 0:64],
                                                1.0, None,
                                                mybir.AluOpType.subtract)
                        nc.sync.dma_start(
                            out=h_loc[t * 128:(t + 1) * 128, :], in_=s0[:])
                    else:
                        nc.sync.dma_start(
                            out=out_d[t * 128:(t + 1) * 128, :],
                            in_=s0[:, 0:64])

            # ============ layer 1 ============
            dense_phase(xs16, NP1, cfg.IN, wa1_sb, xcat1, aldf1)
            ald_gather(aldf1, aldl1)
            edge_sweep(xcat1, NP1, aldl1, 1)

            # ============ exchange ============
            nc.gpsimd.collective_compute(
                "AllGather", mybir.AluOpType.bypass,
                replica_groups=[list(range(cfg.NCORES))],
                ins=[h_loc.opt()], outs=[h_full.opt()])
            nc.gpsimd.dma_start(
                out=h16[:, :].flatten(),
                in_=h_full[:, :].flatten())

            # ============ layer 2 ============
            dense_phase(h16, NP2, cfg.H, wa2_sb, xcat2, aldf2)
            ald_gather(aldf2, aldl2)
            edge_sweep(xcat2, NP2, aldl2, 2)

    nc.compile()
    return nc


def _run(cfg, inputs, run_fn):
    prep = host_prep(cfg, inputs["edge_index"])
    wa1 = _weights_cat(np.asarray(inputs["W1"], np.float32),
                       np.asarray(inputs["a_src1"], np.float32),
                       np.asarray(inputs["a_dst1"], np.float32),
                       cfg.HEADS, cfg.H)
    wa2 = _weights_cat(np.asarray(inputs["W2"], np.float32),
                       np.asarray(inputs["a_src2"], np.float32),
                       np.asarray(inputs["a_dst2"], np.float32),
                       cfg.HEADS, cfg.OUT)
    mconst = np.tile(np.arange(128, dtype=np.float32)[None, :], (128, 1))
    ident = np.eye(128, dtype=np.float32)
    x = np.ascontiguousarray(np.asarray(inputs["x"], np.float32))

    nc = build_kernel(cfg, prep["C_lo"], prep["C_hi"], prep["n_ald"])
    in_maps = []
    for c in range(cfg.NCORES):
        in_maps.append({
            "x": x, "wa1": wa1, "wa2": wa2, "mconst": mconst, "ident": ident,
            "gidx": prep["gidx"][c], "dstloc": prep["dstloc"][c],
            "aldg": prep["aldg"][c],
        })
    results = run_fn(nc, in_maps)
    out = np.concatenate([results[c]["out_slice"]
                          for c in range(cfg.NCORES)], axis=0)
    return out[:cfg.N]


def kernel(**inputs) -> np.ndarray:
    cfg = FULL

    def run_fn(nc, in_maps):
        res = run_bass_kernel_spmd(
            nc, in_maps, core_ids=list(range(cfg.NCORES)),
            tmpdir=os.environ.get("GAT_TMPDIR") or None,
            trace=os.environ.get("GAT_TRACE", "0") == "1")
        if res.exec_time_ns is not None:
            print(f"HW exec time: {res.exec_time_ns} ns")
        return res.results

    return _run(cfg, inputs, run_fn)



# revision 26
# speedup vs baseline: 2.1036x; 2.1036x over previous
"""2-layer GAT (heads=4, concat=False, ELU between) on 8 Trainium2 cores.

v2 strategy (evolved from the project-then-gather baseline):
- Dense phases compute XCAT[n] = [xh(n) bf16 (256) | als(n) f32 | pad] (768B
  rows) for every node; per-edge dma_gather fetches src rows. Same edge
  indices serve both layers.
- Core c owns dst nodes [c*6272, (c+1)*6272). Edges dst-sorted into 128-node
  dst-tiles; per tile C 128-edge chunks (lo/hi table split at node 32768 for
  int16 idx). Gathers use EXACT per-tile counts via Pool-engine registers and
  trailing -1 index padding (padding costs no descriptors / bytes).
- Per tile (bf16 edge pipeline):
  - sel one-hot [128e, C, 128d] built in ONE broadcast is_equal on DVE.
  - selT (dst-major one-hot) shipped precomputed from host; per-chunk
    ald_e = selT_c^T @ ald (one small PE matmul each; no transposes).
  - alpha = als + ald_e; lrelu via scalar_tensor_tensor; capped; ACT exp
    straight to bf16 w.
  - gw = G * w via ONE 4D-broadcast DVE multiply; w cols appended.
  - PSUM-accumulated scatter agg[d, 0:256] += sel_c^T @ gw_c with
    denominators in cols 256:260.
  - head-mean via reciprocal + broadcast multiply on DVE; ELU; store.
- h AllGathered between layers; output assembled on host.
"""
import sys
import os

sys.path.insert(0, '/opt/pypackages')
sys.path.insert(0, '/opt/trn_rl_repo')

import numpy as np
import ml_dtypes

import concourse.bacc as bacc
import concourse.mybir as mybir
import concourse.tile as tile
from concourse.bass_utils import run_bass_kernel_spmd

F16 = mybir.dt.float16
F32 = mybir.dt.float32
BF16 = mybir.dt.bfloat16
I16 = mybir.dt.int16
I32 = mybir.dt.int32

NEG_SLOPE = 0.2
ALPHA_CAP = 60.0

DEBUG_DUMP = False

# Spread SWDGE gathers over the 4 queues for parallel descriptor gen.
# CoreSim locks each SWDGE sem lane to one queue (a sim-only bookkeeping
# constraint the scheduler's reordering makes unsatisfiable), so sim-based
# tests set this False; hardware (per v1 measurements) has no such issue.
QUEUE_SPREAD = True


class Cfg:
    def __init__(self, n, n_in, n_hid, n_out, heads, ncores, tiles_per_core,
                 split):
        self.N = n
        self.IN = n_in
        self.H = n_hid
        self.OUT = n_out
        self.HEADS = heads
        self.NCORES = ncores
        self.T = tiles_per_core              # dst-tiles per core
        self.NPC = tiles_per_core * 128      # nodes per core (padded)
        self.NPAD = ncores * self.NPC        # global padded node count
        self.SPLIT = split                   # int16 gather split boundary
        self.ROW = 384                       # elems per XCAT row (768B)
        self.XH = heads * n_hid              # 256 (=heads*OUT for layer 2)
        assert self.XH == 256 and self.ROW == 384


FULL = Cfg(50000, 128, 64, 64, 4, 8, 49, 32768)


def _wrap16(idx):
    """[n] int array -> [128, n//16] int16 dma_gather layout, replicated x8."""
    n = len(idx)
    assert n % 16 == 0
    base = np.asarray(idx, dtype=np.int16).reshape(n // 16, 16).T  # [16, n/16]
    return np.tile(base, (8, 1))


def host_prep(cfg, edge_index):
    """Build per-core gather indices / dstloc / selT / counts. Returns dict."""
    src = np.asarray(edge_index[0], dtype=np.int64)
    dst = np.asarray(edge_index[1], dtype=np.int64)
    loops = np.arange(cfg.N, dtype=np.int64)
    src = np.concatenate([src, loops])
    dst = np.concatenate([dst, loops])

    core_of = dst // cfg.NPC
    tile_of = (dst % cfg.NPC) // 128

    lists = [[None] * cfg.T for _ in range(cfg.NCORES)]
    c_lo_max = c_hi_max = 1
    order = np.lexsort((src, tile_of, core_of))
    src_s, dst_s = src[order], dst[order]
    key = core_of[order] * cfg.T + tile_of[order]
    starts = np.searchsorted(key, np.arange(cfg.NCORES * cfg.T), side='left')
    ends = np.searchsorted(key, np.arange(cfg.NCORES * cfg.T), side='right')
    for c in range(cfg.NCORES):
        for t in range(cfg.T):
            k = c * cfg.T + t
            s, e = starts[k], ends[k]
            es, ed = src_s[s:e], dst_s[s:e]
            lo = es < cfg.SPLIT
            lists[c][t] = (es[lo], ed[lo], es[~lo], ed[~lo])
            c_lo_max = max(c_lo_max, (len(es[lo]) + 127) // 128)
            c_hi_max = max(c_hi_max, (len(es[~lo]) + 127) // 128)
    C_lo, C_hi = c_lo_max, c_hi_max
    C = C_lo + C_hi

    gidx = np.full((cfg.NCORES, cfg.T, 128, C * 8), -1, dtype=np.int16)
    dstloc = np.full((cfg.NCORES, cfg.T, 128, C), -1.0,
                     dtype=ml_dtypes.bfloat16)
    selt = np.zeros((cfg.NCORES, cfg.T, 128, C * 128),
                    dtype=ml_dtypes.bfloat16)
    cnt = np.zeros((cfg.NCORES, cfg.T, 2), dtype=np.int32)
    dgrid = np.arange(128)[:, None, None]          # [128d, 1, 1]
    for c in range(cfg.NCORES):
        for t in range(cfg.T):
            base = (c * cfg.T + t) * 128
            es_lo, ed_lo, es_hi, ed_hi = lists[c][t]
            n_lo, n_hi = max(len(es_lo), 1), max(len(es_hi), 1)
            ilo = np.full(C_lo * 128, -1, dtype=np.int64)
            ilo[:len(es_lo)] = es_lo
            if len(es_lo) == 0:
                ilo[0] = 0
            ihi = np.full(C_hi * 128, -1, dtype=np.int64)
            ihi[:len(es_hi)] = es_hi - cfg.SPLIT
            if len(es_hi) == 0:
                ihi[0] = 0
            gidx[c, t, :, :C_lo * 8] = _wrap16(ilo)
            gidx[c, t, :, C_lo * 8:] = _wrap16(ihi)
            cnt[c, t] = (n_lo, n_hi)
            dl = np.full((C * 128,), -1.0, dtype=np.float32)
            dl[:len(ed_lo)] = (ed_lo - base).astype(np.float32)
            dl[C_lo * 128:C_lo * 128 + len(ed_hi)] = \
                (ed_hi - base).astype(np.float32)
            dl2 = dl.reshape(C, 128)               # [c, e]
            dstloc[c, t] = dl2.T.astype(ml_dtypes.bfloat16)
            selt[c, t] = (dgrid == dl2[None, :, :]).astype(
                ml_dtypes.bfloat16).reshape(128, C * 128)
    # ald group-gather indices: groups of 16 nodes; per-core pad to 128-mult
    gpc = cfg.NPC // 16  # groups per core
    aldg = np.zeros((cfg.NCORES, 128, (gpc + 127) // 128 * 8), dtype=np.int16)
    n_ald = ((gpc + 127) // 128) * 128
    for c in range(cfg.NCORES):
        g = np.zeros(n_ald, dtype=np.int64)
        g[:gpc] = c * gpc + np.arange(gpc)
        aldg[c] = _wrap16(g)
    return dict(C_lo=C_lo, C_hi=C_hi, C=C, gidx=gidx, dstloc=dstloc,
                selt=selt, cnt=cnt, aldg=aldg, n_ald=n_ald)


def _weights_cat(W, a_src, a_dst, heads, ch):
    """[Fin, heads*ch] + [heads, ch]x2 -> fp16 [Fin, heads*ch + 8]."""
    fin = W.shape[0]
    ws = np.einsum('fhc,hc->fh', W.reshape(fin, heads, ch), a_src)
    wd = np.einsum('fhc,hc->fh', W.reshape(fin, heads, ch), a_dst)
    out = np.zeros((fin, heads * ch + 8), dtype=np.float16)
    out[:, :heads * ch] = W.astype(np.float16)
    out[:, heads * ch:heads * ch + heads] = ws.astype(np.float16)
    out[:, heads * ch + heads:heads * ch + 2 * heads] = wd.astype(np.float16)
    return out


def build_kernel(cfg, C_lo, C_hi, n_ald):
    C = C_lo + C_hi
    nc = bacc.Bacc("TRN2", target_bir_lowering=False, debug=False,
                   num_devices=cfg.NCORES, num_swdge_queues=4)
    NP1 = ((cfg.N + 127) // 128) * 128        # XCAT1 rows
    NP2 = cfg.NPAD                             # XCAT2 rows

    x_in = nc.dram_tensor("x", [cfg.N, cfg.IN], F32, kind="ExternalInput")
    wa1 = nc.dram_tensor("wa1", [cfg.IN, 264], F16, kind="ExternalInput")
    wa2 = nc.dram_tensor("wa2", [cfg.H, 264], F16, kind="ExternalInput")
    mconst = nc.dram_tensor("mconst", [128, C * 128], BF16,
                            kind="ExternalInput")
    gidx_d = nc.dram_tensor("gidx", [cfg.T, 128, C * 8], I16,
                            kind="ExternalInput")
    dstloc_d = nc.dram_tensor("dstloc", [cfg.T, 128, C], BF16,
                              kind="ExternalInput")
    selt_d = nc.dram_tensor("selt", [cfg.T, 128, C * 128], BF16,
                            kind="ExternalInput")
    cnt_d = nc.dram_tensor("cnt", [cfg.T, 2], I32, kind="ExternalInput")
    aldg_d = nc.dram_tensor("aldg", [128, n_ald // 16], I16,
                            kind="ExternalInput")
    out_d = nc.dram_tensor("out_slice", [cfg.NPC, cfg.OUT], F32,
                           kind="ExternalOutput")
    if DEBUG_DUMP:
        dbg_sd = nc.dram_tensor("dbg_sd", [128, C * 128], F32,
                                kind="ExternalOutput")
        dbg_alpha = nc.dram_tensor("dbg_alpha", [128, C * 4], F32,
                                   kind="ExternalOutput")
        dbg_wb = nc.dram_tensor("dbg_wb", [128, C * 4], F32,
                                kind="ExternalOutput")
        dbg_gw = nc.dram_tensor("dbg_gw", [128, C * 260], F32,
                                kind="ExternalOutput")
        dbg_aldps = nc.dram_tensor("dbg_aldps", [128, C * 4], F32,
                                   kind="ExternalOutput")
        dbg_g = nc.dram_tensor("dbg_g", [128, C * 384], F32,
                               kind="ExternalOutput")

    ADD = mybir.AluOpType.add
    MULT = mybir.AluOpType.mult
    MAXOP = mybir.AluOpType.max
    MINOP = mybir.AluOpType.min
    SUB = mybir.AluOpType.subtract
    ISEQ = mybir.AluOpType.is_equal

    with tile.TileContext(nc) as tc:
        with tc.tile_pool(name="dram", bufs=1, space="DRAM") as dpool, \
             tc.tile_pool(name="const", bufs=1) as cpool, \
             tc.tile_pool(name="work", bufs=2) as pool, \
             tc.tile_pool(name="ld", bufs=3) as ldpool, \
             tc.tile_pool(name="gpool", bufs=3) as gpool, \
             tc.tile_pool(name="stp", bufs=2) as stpool, \
             tc.tile_pool(name="sdp", bufs=2) as sdpool, \
             tc.tile_pool(name="gw", bufs=2) as gwpool, \
             tc.tile_pool(name="wp", bufs=2) as wpool, \
             tc.tile_pool(name="bp", bufs=2) as bpool, \
             tc.tile_pool(name="psA", bufs=2, space="PSUM") as psA, \
             tc.tile_pool(name="psAgg", bufs=2, space="PSUM") as psAgg, \
             tc.tile_pool(name="psAld", bufs=2, space="PSUM") as psAld:

            xs16 = dpool.tile([NP1, cfg.IN], F16, name="xs16", uniquify=False)
            xcat1 = dpool.tile([NP1, cfg.ROW], BF16, name="xcat1",
                               uniquify=False)
            aldf1 = dpool.tile([NP2, 4], F32, name="aldf1", uniquify=False)
            aldl1 = dpool.tile([n_ald * 16, 4], F32, name="aldl1",
                               uniquify=False)
            h_loc = dpool.tile([cfg.NPC, 128], F32, name="h_loc",
                               uniquify=False)
            h_full = dpool.tile([NP2, 128], F32, name="h_full",
                                uniquify=False, addr_space="Shared")
            h16 = dpool.tile([NP2, 128], F16, name="h16", uniquify=False)
            xcat2 = dpool.tile([NP2, cfg.ROW], BF16, name="xcat2",
                               uniquify=False)
            aldf2 = dpool.tile([NP2, 4], F32, name="aldf2", uniquify=False)
            aldl2 = dpool.tile([n_ald * 16, 4], F32, name="aldl2",
                               uniquify=False)

            mconst_sb = cpool.tile([128, C * 128], BF16)
            nc.sync.dma_start(out=mconst_sb[:], in_=mconst[:, :])
            wa1_sb = cpool.tile([cfg.IN, 264], F16)
            nc.sync.dma_start(out=wa1_sb[:], in_=wa1[:, :])
            wa2_sb = cpool.tile([cfg.H, 264], F16)
            nc.sync.dma_start(out=wa2_sb[:], in_=wa2[:, :])
            aldg_sb = cpool.tile([128, n_ald // 16], I16)
            nc.sync.dma_start(out=aldg_sb[:], in_=aldg_d[:, :])
            cnt_sb = cpool.tile([1, cfg.T * 2], I32)
            nc.sync.dma_start(out=cnt_sb[:],
                              in_=cnt_d[:, :].rearrange("t k -> (t k)")
                              .unsqueeze(0))
            zero_sb = cpool.tile([128, 128], F16)
            nc.gpsimd.memset(zero_sb[:], 0)
            zero_f32 = cpool.tile([128, 8], F32)
            nc.gpsimd.memset(zero_f32[:], 0)

            # gather count registers: 4-deep rotation x (lo, hi)
            regs = [(nc.gpsimd.alloc_register(f"cl{q}"),
                     nc.gpsimd.alloc_register(f"ch{q}")) for q in range(4)]

            # Pool-DMA position counter. The tile scheduler assigns SWDGE
            # sem lanes round-robin (8 lanes) over Pool DMA instructions in
            # issue order, and each lane is bound to one SWDGE queue. Keep
            # queue_num = position % 4 for gathers, and only issue
            # forced-queue-0 Pool dma_starts at positions % 4 == 0.
            pctr = [0]

            def q_next():
                q = pctr[0] % 4 if QUEUE_SPREAD else 0
                pctr[0] += 1
                return q

            zidx = cpool.tile([128, 8], I16)
            nc.gpsimd.memset(zidx[:], 0)
            dummy_g = cpool.tile([128, 384], BF16)
            zero_wide = cpool.tile([128, C * cfg.ROW], BF16)
            nc.gpsimd.memset(zero_wide[:], 0)

            def align_pool_q0(table):
                while QUEUE_SPREAD and pctr[0] % 4 != 0:
                    nc.gpsimd.dma_gather(
                        dummy_g[:].unsqueeze(1), table[0:128, :], zidx[:],
                        128, 128, cfg.ROW, single_packet=False,
                        queue_num=q_next())

            # ---- stage x -> fp16, zero pad rows ----
            assert pctr[0] % 4 == 0
            pctr[0] += 1  # Pool dma_start below rides queue 0
            nc.gpsimd.dma_start(
                out=xs16[0:cfg.N, :].flatten(),
                in_=x_in[:, :].flatten())
            if NP1 > cfg.N:
                npad = NP1 - cfg.N
                nc.sync.dma_start(out=xs16[cfg.N:NP1, :],
                                  in_=zero_sb[0:npad, 0:cfg.IN])
            if NP2 > NP1:
                nc.sync.dma_start(out=aldf1[NP1:NP2, :],
                                  in_=zero_f32[0:NP2 - NP1, 0:4])

            def dense_phase(src16, n_rows, fin, wa_sb, xcat, aldf):
                """src16 [n_rows, fin fp16] @ wa -> xcat (bf16) + aldf."""
                BT = 8  # subtiles per batch
                B = BT * 128
                nb = 0
                bi = 0
                while nb < n_rows:
                    bsz = min(B, n_rows - nb)
                    st = bsz // 128
                    xT = pool.tile([128, B], F16, name=f"xT{id(xcat)}_{bi}",
                                   tag="xT")
                    nc.sync.dma_start(out=xT[:, 0:bsz],
                                      in_=src16[nb:nb + bsz, :],
                                      transpose=True)
                    xc = pool.tile([128, BT, 264], BF16,
                                   name=f"xc{id(xcat)}_{bi}", tag="xc")
                    xcf = xc[:].bitcast(F32)  # [128, BT, 132]
                    arow = pool.tile([128, BT, 4], F32,
                                     name=f"ar{id(xcat)}_{bi}", tag="ar")
                    for s in range(st):
                        ps = psA.tile([128, 264], F32, name=f"dps{bi}_{s}",
                                      tag="dps")
                        nc.tensor.matmul(
                            ps[:], xT[0:fin, s * 128:(s + 1) * 128],
                            wa_sb[:], start=True, stop=True)
                        nc.scalar.activation(
                            xc[:, s, 0:256], ps[:, 0:256],
                            mybir.ActivationFunctionType.Copy)
                        nc.vector.tensor_copy(xcf[:, s, 128:132],
                                              ps[:, 256:260])
                        nc.vector.tensor_copy(arow[:, s, :], ps[:, 260:264])
                    nc.sync.dma_start(
                        out=xcat[nb:nb + bsz, 0:264].rearrange(
                            "(s p) d -> p s d", p=128),
                        in_=xc[:, 0:st, :])
                    nc.sync.dma_start(
                        out=aldf[nb:nb + bsz, :].rearrange(
                            "(s p) d -> p s d", p=128),
                        in_=arow[:, 0:st, :])
                    nb += bsz
                    bi += 1

            def ald_gather(aldf, aldl):
                asb = pool.tile([128, n_ald // 128, 64], F32, tag="asb")
                nc.gpsimd.dma_gather(
                    asb[:],
                    aldf[:, :].rearrange("(g k) d -> g (k d)", k=16),
                    aldg_sb[:], n_ald, n_ald, 64, single_packet=False,
                    queue_num=q_next())
                nc.sync.dma_start(
                    out=aldl[:, :].rearrange("(c p j) d -> p c (j d)",
                                             p=128, j=16),
                    in_=asb[:])

            def sweep_a(xcat, n_rows, aldl, layer, t):
                """Per-tile stage A: loads, gathers, sel, ald, alpha, gw."""
                sfx = f"_{layer}_{t}"
                q = t % 4
                idx_t = ldpool.tile([128, C * 8], I16, name="ix" + sfx,
                                    tag="ix")
                nc.sync.dma_start(out=idx_t[:], in_=gidx_d[t, :, :])
                dst_t = ldpool.tile([128, C], BF16, name="dl" + sfx, tag="dl")
                nc.sync.dma_start(out=dst_t[:], in_=dstloc_d[t, :, :])
                selt_t = stpool.tile([128, C * 128], BF16, name="sT" + sfx,
                                     tag="sT")
                nc.sync.dma_start(out=selt_t[:], in_=selt_d[t, :, :])
                ald32 = ldpool.tile([128, 4], F32, name="at" + sfx, tag="at")
                nc.sync.dma_start(out=ald32[:],
                                  in_=aldl[t * 128:(t + 1) * 128, :])
                ald16 = ldpool.tile([128, 4], BF16, name="a6" + sfx, tag="a6")
                nc.vector.tensor_copy(ald16[:], ald32[:])

                r_lo, r_hi = regs[q]
                nc.gpsimd.reg_load(r_lo, cnt_sb[0:1, 2 * t:2 * t + 1])
                nc.gpsimd.reg_load(r_hi, cnt_sb[0:1, 2 * t + 1:2 * t + 2])
                G = gpool.tile([128, C, cfg.ROW], BF16, name="G" + sfx,
                               tag="G")
                # zero first: -1-padded gather slots are skipped, and stale
                # bytes would poison agg via 0 x NaN in the scatter matmul
                nc.scalar.activation(
                    G[:].rearrange("p c f -> p (c f)"), zero_wide[:],
                    mybir.ActivationFunctionType.Copy)
                nc.gpsimd.dma_gather(
                    G[:, 0:C_lo, :], xcat[0:cfg.SPLIT, :],
                    idx_t[:, 0:C_lo * 8], C_lo * 128, r_lo,
                    cfg.ROW, single_packet=False, queue_num=q_next())
                nc.gpsimd.dma_gather(
                    G[:, C_lo:C, :], xcat[cfg.SPLIT:n_rows, :],
                    idx_t[:, C_lo * 8:], C_hi * 128, r_hi,
                    cfg.ROW, single_packet=False, queue_num=q_next())
                Gf = G[:].bitcast(F32)  # [128, C, 192]

                # one-hot (edge-major) for the agg matmul stationaries
                sd = sdpool.tile([128, C, 128], BF16, name="sd" + sfx,
                                 tag="sd")
                nc.vector.tensor_tensor(
                    out=sd[:],
                    in0=mconst_sb[:].rearrange("p (c j) -> p c j", j=128),
                    in1=dst_t[:].unsqueeze(2).broadcast_to([128, C, 128]),
                    op=ISEQ)

                # per-edge ald via host selT: ald_e = selT_c^T @ ald16
                ald_ps = psAld.tile([128, C * 4], F32, name="alp" + sfx,
                                    tag="alp")
                sT3 = selt_t[:].rearrange("p (c j) -> p c j", j=128)
                for c in range(C):
                    nc.tensor.matmul(
                        ald_ps[:, c * 4:(c + 1) * 4], sT3[:, c, :], ald16[:],
                        start=True, stop=True, skip_group_check=True)

                alpha = wpool.tile([128, C, 4], F32, name="alf" + sfx,
                                   tag="alf")
                nc.vector.tensor_tensor(
                    out=alpha[:], in0=Gf[:, :, 128:132],
                    in1=ald_ps[:].rearrange("p (c f) -> p c f", f=4),
                    op=ADD)
                alr = wpool.tile([128, C, 4], F32, name="alr" + sfx,
                                 tag="alr")
                nc.vector.scalar_tensor_tensor(
                    out=alr[:], in0=alpha[:], scalar=NEG_SLOPE, in1=alpha[:],
                    op0=MULT, op1=MAXOP)
                nc.vector.tensor_scalar(alr[:], alr[:], ALPHA_CAP, None,
                                        MINOP)
                wb = wpool.tile([128, C * 4], BF16, name="wb" + sfx, tag="wb")
                nc.scalar.activation(
                    wb[:].rearrange("p (c f) -> p c f", f=4), alr[:],
                    mybir.ActivationFunctionType.Exp)

                gw = gwpool.tile([128, C, 260], BF16, name="gw" + sfx,
                                 tag="gw")
                nc.vector.tensor_tensor(
                    out=gw[:, :, 0:256].rearrange("p c (h f) -> p c h f",
                                                  f=64),
                    in0=G[:, :, 0:256].rearrange("p c (h f) -> p c h f",
                                                 f=64),
                    in1=wb[:].rearrange("p (c h) -> p c h", h=4).unsqueeze(3)
                        .broadcast_to([128, C, 4, 64]),
                    op=MULT)
                nc.vector.tensor_copy(
                    gw[:, :, 256:260],
                    wb[:].rearrange("p (c h) -> p c h", h=4))
                agg = psAgg.tile([128, 260], F32, name="agg" + sfx, tag="agg")
                if DEBUG_DUMP and layer == 1 and t == 0:
                    for dt_, src_ap in [
                            (dbg_sd, sd[:].rearrange("p c j -> p (c j)")),
                            (dbg_alpha,
                             alpha[:].rearrange("p c f -> p (c f)")),
                            (dbg_wb, wb[:]),
                            (dbg_gw, gw[:].rearrange("p c f -> p (c f)")),
                            (dbg_aldps, ald_ps[:]),
                            (dbg_g, G[:].rearrange("p c f -> p (c f)"))]:
                        tmpd = wpool.tile(list(dt_.shape), F32,
                                          name=f"dbg{dt_.name}", tag="dbg",
                                          bufs=1)
                        nc.vector.tensor_copy(tmpd[:], src_ap)
                        nc.sync.dma_start(out=dt_[:, :], in_=tmpd[:])
                return dict(sd=sd, gw=gw, agg=agg, t=t)

            def sweep_b(st_dict, layer):
                """Per-tile stage B: agg matmuls, normalize, ELU/store."""
                t = st_dict["t"]
                sfx = f"_{layer}_{t}"
                sd, gw, agg = st_dict["sd"], st_dict["gw"], st_dict["agg"]
                for c in range(C):
                    nc.tensor.matmul(
                        agg[:, 0:260], sd[:, c, :], gw[:, c, :],
                        start=(c == 0), stop=(c == C - 1),
                        skip_group_check=True)
                den = bpool.tile([128, 4], F32, name="dn" + sfx, tag="dn")
                nc.vector.tensor_scalar(den[:], agg[:, 256:260], 4.0, 4e-16,
                                        MULT, MAXOP)
                rec = bpool.tile([128, 4], F32, name="rc" + sfx, tag="rc")
                nc.vector.reciprocal(rec[:], den[:])
                tmp = bpool.tile([128, 4, 64], F32, name="tm" + sfx,
                                 tag="tm")
                nc.vector.tensor_tensor(
                    out=tmp[:],
                    in0=agg[:, 0:256].rearrange("p (h f) -> p h f", f=64),
                    in1=rec[:].unsqueeze(2).broadcast_to([128, 4, 64]),
                    op=MULT)
                s0 = bpool.tile([128, 64], F32, name="s0" + sfx, tag="s0")
                s1 = bpool.tile([128, 64], F32, name="s1" + sfx, tag="s1")
                nc.vector.tensor_tensor(out=s0[:], in0=tmp[:, 0, :],
                                        in1=tmp[:, 1, :], op=ADD)
                nc.vector.tensor_tensor(out=s1[:], in0=tmp[:, 2, :],
                                        in1=tmp[:, 3, :], op=ADD)
                nc.vector.tensor_tensor(out=s0[:], in0=s0[:], in1=s1[:],
                                        op=ADD)
                if layer == 1:
                    # ELU(s) = max(s,0) + exp(min(s,0)) - 1
                    ng = bpool.tile([128, 64], F32, name="ng" + sfx,
                                    tag="ng")
                    nc.vector.tensor_scalar(ng[:], s0[:], 0.0, None, MINOP)
                    ex = bpool.tile([128, 64], F32, name="ex" + sfx,
                                    tag="ex")
                    nc.scalar.activation(
                        ex[:], ng[:], mybir.ActivationFunctionType.Exp)
                    nc.vector.tensor_scalar(s0[:], s0[:], 0.0, None, MAXOP)
                    nc.vector.tensor_tensor(out=s0[:], in0=s0[:], in1=ex[:],
                                            op=ADD)
                    nc.vector.tensor_scalar(s0[:], s0[:], 1.0, None, SUB)
                    nc.sync.dma_start(
                        out=h_loc[t * 128:(t + 1) * 128, 0:64], in_=s0[:])
                else:
                    nc.sync.dma_start(
                        out=out_d[t * 128:(t + 1) * 128, :], in_=s0[:])

            def edge_sweep(xcat, n_rows, aldl, layer):
                prev = None
                for t in range(cfg.T):
                    cur = sweep_a(xcat, n_rows, aldl, layer, t)
                    if prev is not None:
                        sweep_b(prev, layer)
                    prev = cur
                sweep_b(prev, layer)

            # ============ layer 1 ============
            dense_phase(xs16, NP1, cfg.IN, wa1_sb, xcat1, aldf1)
            ald_gather(aldf1, aldl1)
            edge_sweep(xcat1, NP1, aldl1, 1)

            # ============ exchange ============
            nc.gpsimd.collective_compute(
                "AllGather", mybir.AluOpType.bypass,
                replica_groups=[list(range(cfg.NCORES))],
                ins=[h_loc.opt()], outs=[h_full.opt()])
            align_pool_q0(xcat1)
            pctr[0] += 1  # Pool dma_start below rides queue 0
            nc.gpsimd.dma_start(
                out=h16[:, :].flatten(),
                in_=h_full[:, :].flatten())

            # ============ layer 2 ============
            dense_phase(h16, NP2, cfg.H, wa2_sb, xcat2, aldf2)
            ald_gather(aldf2, aldl2)
            edge_sweep(xcat2, NP2, aldl2, 2)

    nc.compile()
    return nc


def _run(cfg, inputs, run_fn):
    prep = host_prep(cfg, inputs["edge_index"])
    C = prep["C"]
    wa1 = _weights_cat(np.asarray(inputs["W1"], np.float32),
                       np.asarray(inputs["a_src1"], np.float32),
                       np.asarray(inputs["a_dst1"], np.float32),
                       cfg.HEADS, cfg.H)
    wa2 = _weights_cat(np.asarray(inputs["W2"], np.float32),
                       np.asarray(inputs["a_src2"], np.float32),
                       np.asarray(inputs["a_dst2"], np.float32),
                       cfg.HEADS, cfg.OUT)
    mconst = np.tile(np.arange(128, dtype=np.float32), (128, C)).astype(
        ml_dtypes.bfloat16)
    x = np.ascontiguousarray(np.asarray(inputs["x"], np.float32))

    nc = build_kernel(cfg, prep["C_lo"], prep["C_hi"], prep["n_ald"])
    in_maps = []
    for c in range(cfg.NCORES):
        in_maps.append({
            "x": x, "wa1": wa1, "wa2": wa2, "mconst": mconst,
            "gidx": prep["gidx"][c], "dstloc": prep["dstloc"][c],
            "selt": prep["selt"][c], "cnt": prep["cnt"][c],
            "aldg": prep["aldg"][c],
        })
    results = run_fn(nc, in_maps)
    out = np.concatenate([results[c]["out_slice"]
                          for c in range(cfg.NCORES)], axis=0)
    return out[:cfg.N]


def kernel(**inputs) -> np.ndarray:
    cfg = FULL

    def run_fn(nc, in_maps):
        res = run_bass_kernel_spmd(
            nc, in_maps, core_ids=list(range(cfg.NCORES)),
            tmpdir=os.environ.get("GAT_TMPDIR") or None,
            trace=os.environ.get("GAT_TRACE", "0") == "1")
        if res.exec_time_ns is not None:
            print(f"HW exec time: {res.exec_time_ns} ns")
        return res.results

    return _run(cfg, inputs, run_fn)


# revision 42
# speedup vs baseline: 2.4161x; 1.1485x over previous
"""2-layer GAT (heads=4, concat=False, ELU between) on 8 Trainium2 cores.

v2 strategy (evolved from the project-then-gather baseline):
- Dense phases compute XCAT[n] = [xh(n) bf16 (256) | als(n) f32 | pad] (768B
  rows) for every node; per-edge dma_gather fetches src rows. Same edge
  indices serve both layers.
- Core c owns dst nodes [c*6272, (c+1)*6272). Edges dst-sorted into 128-node
  dst-tiles; per tile C 128-edge chunks (lo/hi table split at node 32768 for
  int16 idx). Gathers use EXACT per-tile counts via Pool-engine registers and
  trailing -1 index padding (padding costs no descriptors / bytes).
- Per tile (bf16 edge pipeline):
  - sel one-hot [128e, C, 128d] built in ONE broadcast is_equal on DVE.
  - selT (dst-major one-hot) shipped precomputed from host; per-chunk
    ald_e = selT_c^T @ ald (one small PE matmul each; no transposes).
  - alpha = als + ald_e; lrelu via scalar_tensor_tensor; capped; ACT exp
    straight to bf16 w.
  - gw = G * w via ONE 4D-broadcast DVE multiply; w cols appended.
  - PSUM-accumulated scatter agg[d, 0:256] += sel_c^T @ gw_c with
    denominators in cols 256:260.
  - head-mean via reciprocal + broadcast multiply on DVE; ELU; store.
- h AllGathered between layers; output assembled on host.
"""
import sys
import os

sys.path.insert(0, '/opt/pypackages')
sys.path.insert(0, '/opt/trn_rl_repo')

import numpy as np
import ml_dtypes

import concourse.bacc as bacc
import concourse.mybir as mybir
import concourse.tile as tile
from concourse.bass_utils import run_bass_kernel_spmd

F16 = mybir.dt.float16
F32 = mybir.dt.float32
BF16 = mybir.dt.bfloat16
I16 = mybir.dt.int16
I32 = mybir.dt.int32

NEG_SLOPE = 0.2
ALPHA_CAP = 60.0

DEBUG_DUMP = False
SINGLE_PACKET = os.environ.get("GAT_SP", "0") == "1"

# Spread SWDGE gathers over the 4 queues for parallel descriptor gen.
# CoreSim locks each SWDGE sem lane to one queue (a sim-only bookkeeping
# constraint the scheduler's reordering makes unsatisfiable), so sim-based
# tests set this False; hardware (per v1 measurements) has no such issue.
QUEUE_SPREAD = True


class Cfg:
    def __init__(self, n, n_in, n_hid, n_out, heads, ncores, tiles_per_core,
                 split):
        self.N = n
        self.IN = n_in
        self.H = n_hid
        self.OUT = n_out
        self.HEADS = heads
        self.NCORES = ncores
        self.T = tiles_per_core              # dst-tiles per core
        self.NPC = tiles_per_core * 128      # nodes per core (padded)
        self.NPAD = ncores * self.NPC        # global padded node count
        self.SPLIT = split                   # int16 gather split boundary
        self.ROW = 384                       # elems per XCAT row (768B)
        self.XH = heads * n_hid              # 256 (=heads*OUT for layer 2)
        assert self.XH == 256 and self.ROW == 384


FULL = Cfg(50000, 128, 64, 64, 4, 8, 49, 32768)


def _wrap16(idx):
    """[n] int array -> [128, n//16] int16 dma_gather layout, replicated x8."""
    n = len(idx)
    assert n % 16 == 0
    base = np.asarray(idx, dtype=np.int16).reshape(n // 16, 16).T  # [16, n/16]
    return np.tile(base, (8, 1))


def host_prep(cfg, edge_index):
    """Build per-core gather indices / dstloc / selT / counts. Returns dict."""
    src = np.asarray(edge_index[0], dtype=np.int64)
    dst = np.asarray(edge_index[1], dtype=np.int64)
    loops = np.arange(cfg.N, dtype=np.int64)
    src = np.concatenate([src, loops])
    dst = np.concatenate([dst, loops])

    core_of = dst // cfg.NPC
    tile_of = (dst % cfg.NPC) // 128

    lists = [[None] * cfg.T for _ in range(cfg.NCORES)]
    c_lo_max = c_hi_max = 1
    order = np.lexsort((src, tile_of, core_of))
    src_s, dst_s = src[order], dst[order]
    key = core_of[order] * cfg.T + tile_of[order]
    starts = np.searchsorted(key, np.arange(cfg.NCORES * cfg.T), side='left')
    ends = np.searchsorted(key, np.arange(cfg.NCORES * cfg.T), side='right')
    for c in range(cfg.NCORES):
        for t in range(cfg.T):
            k = c * cfg.T + t
            s, e = starts[k], ends[k]
            es, ed = src_s[s:e], dst_s[s:e]
            lo = es < cfg.SPLIT
            lists[c][t] = (es[lo], ed[lo], es[~lo], ed[~lo])
            c_lo_max = max(c_lo_max, (len(es[lo]) + 127) // 128)
            c_hi_max = max(c_hi_max, (len(es[~lo]) + 127) // 128)
    C_lo, C_hi = c_lo_max, c_hi_max
    C = C_lo + C_hi

    gidx = np.full((cfg.NCORES, cfg.T, 128, C * 8), -1, dtype=np.int16)
    selt = np.zeros((cfg.NCORES, cfg.T, 128, C * 128),
                    dtype=ml_dtypes.bfloat16)
    seld = np.zeros((cfg.NCORES, cfg.T, 128, C * 128),
                    dtype=ml_dtypes.bfloat16)
    cnt = np.zeros((cfg.NCORES, cfg.T, 2), dtype=np.int32)
    dgrid = np.arange(128)[:, None, None]          # [128d, 1, 1]
    for c in range(cfg.NCORES):
        for t in range(cfg.T):
            base = (c * cfg.T + t) * 128
            es_lo, ed_lo, es_hi, ed_hi = lists[c][t]
            n_lo, n_hi = max(len(es_lo), 1), max(len(es_hi), 1)
            ilo = np.full(C_lo * 128, -1, dtype=np.int64)
            ilo[:len(es_lo)] = es_lo
            if len(es_lo) == 0:
                ilo[0] = 0
            ihi = np.full(C_hi * 128, -1, dtype=np.int64)
            ihi[:len(es_hi)] = es_hi - cfg.SPLIT
            if len(es_hi) == 0:
                ihi[0] = 0
            gidx[c, t, :, :C_lo * 8] = _wrap16(ilo)
            gidx[c, t, :, C_lo * 8:] = _wrap16(ihi)
            cnt[c, t] = (n_lo, n_hi)
            dl = np.full((C * 128,), -1.0, dtype=np.float32)
            dl[:len(ed_lo)] = (ed_lo - base).astype(np.float32)
            dl[C_lo * 128:C_lo * 128 + len(ed_hi)] = \
                (ed_hi - base).astype(np.float32)
            dl2 = dl.reshape(C, 128)               # [c, e]
            selt[c, t] = (dgrid == dl2[None, :, :]).astype(
                ml_dtypes.bfloat16).reshape(128, C * 128)
            seld[c, t] = (dl2.T[:, :, None] ==
                          np.arange(128)[None, None, :]).astype(
                ml_dtypes.bfloat16).reshape(128, C * 128)
    # ald group-gather indices: groups of 16 nodes; per-core pad to 128-mult
    gpc = cfg.NPC // 16  # groups per core
    aldg = np.zeros((cfg.NCORES, 128, (gpc + 127) // 128 * 8), dtype=np.int16)
    n_ald = ((gpc + 127) // 128) * 128
    for c in range(cfg.NCORES):
        g = np.zeros(n_ald, dtype=np.int64)
        g[:gpc] = c * gpc + np.arange(gpc)
        aldg[c] = _wrap16(g)
    return dict(C_lo=C_lo, C_hi=C_hi, C=C, gidx=gidx,
                selt=selt, seld=seld, cnt=cnt, aldg=aldg, n_ald=n_ald)


def _weights_cat(W, a_src, a_dst, heads, ch):
    """[Fin, heads*ch] + [heads, ch]x2 -> fp16 [Fin, heads*ch + 8]."""
    fin = W.shape[0]
    ws = np.einsum('fhc,hc->fh', W.reshape(fin, heads, ch), a_src)
    wd = np.einsum('fhc,hc->fh', W.reshape(fin, heads, ch), a_dst)
    out = np.zeros((fin, heads * ch + 8), dtype=np.float16)
    out[:, :heads * ch] = W.astype(np.float16)
    out[:, heads * ch:heads * ch + heads] = ws.astype(np.float16)
    out[:, heads * ch + heads:heads * ch + 2 * heads] = wd.astype(np.float16)
    return out


def build_kernel(cfg, C_lo, C_hi, n_ald):
    C = C_lo + C_hi
    nc = bacc.Bacc("TRN2", target_bir_lowering=False, debug=False,
                   num_devices=cfg.NCORES, num_swdge_queues=4)
    NP1 = ((cfg.N + 127) // 128) * 128        # XCAT1 rows
    NP2 = cfg.NPAD                             # XCAT2 rows

    x_in = nc.dram_tensor("x", [cfg.N, cfg.IN], F32, kind="ExternalInput")
    wa1 = nc.dram_tensor("wa1", [cfg.IN, 264], F16, kind="ExternalInput")
    wa2 = nc.dram_tensor("wa2", [cfg.H, 264], F16, kind="ExternalInput")
    gidx_d = nc.dram_tensor("gidx", [cfg.T, 128, C * 8], I16,
                            kind="ExternalInput")
    selt_d = nc.dram_tensor("selt", [cfg.T, 128, C * 128], BF16,
                            kind="ExternalInput")
    seld_d = nc.dram_tensor("seld", [cfg.T, 128, C * 128], BF16,
                            kind="ExternalInput")
    cnt_d = nc.dram_tensor("cnt", [cfg.T, 2], I32, kind="ExternalInput")
    aldg_d = nc.dram_tensor("aldg", [128, n_ald // 16], I16,
                            kind="ExternalInput")
    out_d = nc.dram_tensor("out_slice", [cfg.NPC, cfg.OUT], F32,
                           kind="ExternalOutput")
    if DEBUG_DUMP:
        dbg_sd = nc.dram_tensor("dbg_sd", [128, C * 128], F32,
                                kind="ExternalOutput")
        dbg_alpha = nc.dram_tensor("dbg_alpha", [128, C * 4], F32,
                                   kind="ExternalOutput")
        dbg_wb = nc.dram_tensor("dbg_wb", [128, C * 4], F32,
                                kind="ExternalOutput")
        dbg_gw = nc.dram_tensor("dbg_gw", [128, C * 260], F32,
                                kind="ExternalOutput")
        dbg_aldps = nc.dram_tensor("dbg_aldps", [128, C * 4], F32,
                                   kind="ExternalOutput")
        dbg_g = nc.dram_tensor("dbg_g", [128, C * 384], F32,
                               kind="ExternalOutput")

    ADD = mybir.AluOpType.add
    MULT = mybir.AluOpType.mult
    MAXOP = mybir.AluOpType.max
    MINOP = mybir.AluOpType.min
    SUB = mybir.AluOpType.subtract
    ISEQ = mybir.AluOpType.is_equal

    with tile.TileContext(nc) as tc:
        with tc.tile_pool(name="dram", bufs=1, space="DRAM") as dpool, \
             tc.tile_pool(name="const", bufs=1) as cpool, \
             tc.tile_pool(name="work", bufs=2) as pool, \
             tc.tile_pool(name="ld", bufs=3) as ldpool, \
             tc.tile_pool(name="gpool", bufs=3) as gpool, \
             tc.tile_pool(name="stp", bufs=2) as stpool, \
             tc.tile_pool(name="sdp", bufs=2) as sdpool, \
             tc.tile_pool(name="gw", bufs=2) as gwpool, \
             tc.tile_pool(name="wp", bufs=2) as wpool, \
             tc.tile_pool(name="bp", bufs=2) as bpool, \
             tc.tile_pool(name="psA", bufs=2, space="PSUM") as psA, \
             tc.tile_pool(name="psAgg", bufs=2, space="PSUM") as psAgg, \
             tc.tile_pool(name="psAld", bufs=2, space="PSUM") as psAld:

            xs16 = dpool.tile([NP1, cfg.IN], F16, name="xs16", uniquify=False)
            xcat1 = dpool.tile([NP1, cfg.ROW], BF16, name="xcat1",
                               uniquify=False)
            aldf1 = dpool.tile([NP2, 4], F32, name="aldf1", uniquify=False)
            aldl1 = dpool.tile([n_ald * 16, 4], F32, name="aldl1",
                               uniquify=False)
            h_loc = dpool.tile([cfg.NPC, 128], F32, name="h_loc",
                               uniquify=False)
            h_full = dpool.tile([NP2, 128], F32, name="h_full",
                                uniquify=False, addr_space="Shared")
            h16 = dpool.tile([NP2, 128], F16, name="h16", uniquify=False)
            xcat2 = dpool.tile([NP2, cfg.ROW], BF16, name="xcat2",
                               uniquify=False)
            aldf2 = dpool.tile([NP2, 4], F32, name="aldf2", uniquify=False)
            aldl2 = dpool.tile([n_ald * 16, 4], F32, name="aldl2",
                               uniquify=False)

            wa1_sb = cpool.tile([cfg.IN, 264], F16)
            nc.sync.dma_start(out=wa1_sb[:], in_=wa1[:, :])
            wa2_sb = cpool.tile([cfg.H, 264], F16)
            nc.sync.dma_start(out=wa2_sb[:], in_=wa2[:, :])
            aldg_sb = cpool.tile([128, n_ald // 16], I16)
            nc.sync.dma_start(out=aldg_sb[:], in_=aldg_d[:, :])
            cnt_sb = cpool.tile([1, cfg.T * 2], I32)
            nc.sync.dma_start(out=cnt_sb[:],
                              in_=cnt_d[:, :].rearrange("t k -> (t k)")
                              .unsqueeze(0))
            zero_sb = cpool.tile([128, 128], F16)
            nc.gpsimd.memset(zero_sb[:], 0)
            zero_f32 = cpool.tile([128, 8], F32)
            nc.gpsimd.memset(zero_f32[:], 0)

            # gather count registers: 4-deep rotation x (lo, hi)
            regs = [(nc.gpsimd.alloc_register(f"cl{q}"),
                     nc.gpsimd.alloc_register(f"ch{q}")) for q in range(4)]

            # Pool-DMA position counter. The tile scheduler assigns SWDGE
            # sem lanes round-robin (8 lanes) over Pool DMA instructions in
            # issue order, and each lane is bound to one SWDGE queue. Keep
            # queue_num = position % 4 for gathers, and only issue
            # forced-queue-0 Pool dma_starts at positions % 4 == 0.
            pctr = [0]

            def q_next():
                q = pctr[0] % 4 if QUEUE_SPREAD else 0
                pctr[0] += 1
                return q

            zidx = cpool.tile([128, 8], I16)
            nc.gpsimd.memset(zidx[:], 0)
            dummy_g = cpool.tile([128, 384], BF16)
            zero_wide = cpool.tile([128, C * cfg.ROW], BF16)
            nc.gpsimd.memset(zero_wide[:], 0)

            def align_pool_q0(table):
                while QUEUE_SPREAD and pctr[0] % 4 != 0:
                    nc.gpsimd.dma_gather(
                        dummy_g[:].unsqueeze(1), table[0:128, :], zidx[:],
                        128, 128, cfg.ROW, single_packet=False,
                        queue_num=q_next())

            # ---- stage x -> fp16, zero pad rows ----
            assert pctr[0] % 4 == 0
            pctr[0] += 1  # Pool dma_start below rides queue 0
            nc.gpsimd.dma_start(
                out=xs16[0:cfg.N, :].flatten(),
                in_=x_in[:, :].flatten())
            if NP1 > cfg.N:
                npad = NP1 - cfg.N
                nc.sync.dma_start(out=xs16[cfg.N:NP1, :],
                                  in_=zero_sb[0:npad, 0:cfg.IN])
            if NP2 > NP1:
                nc.sync.dma_start(out=aldf1[NP1:NP2, :],
                                  in_=zero_f32[0:NP2 - NP1, 0:4])

            def dense_phase(src16, n_rows, fin, wa_sb, xcat, aldf):
                """src16 [n_rows, fin fp16] @ wa -> xcat (bf16) + aldf.

                Nodes are interleaved stride-BT across subtiles so each
                partition owns BT consecutive DRAM rows: the xcat/aldf
                writes become one contiguous BT*768B / BT*16B chunk per
                partition instead of per-row packets."""
                BT = 8  # subtiles per batch
                B = BT * 128
                nb = 0
                bi = 0
                while nb < n_rows:
                    bsz = min(B, n_rows - nb)
                    assert bsz % BT == 0
                    pc = bsz // BT  # partitions used
                    xT = pool.tile([128, B], F16, name=f"xT{id(xcat)}_{bi}",
                                   tag="xT")
                    nc.sync.dma_start(out=xT[:, 0:bsz],
                                      in_=src16[nb:nb + bsz, :],
                                      transpose=True)
                    xTs = xT[0:fin, 0:bsz].rearrange("f (p s) -> f s p", s=BT)
                    xc = pool.tile([128, BT, cfg.ROW], BF16,
                                   name=f"xc{id(xcat)}_{bi}", tag="xc")
                    nc.scalar.activation(
                        xc[:, :, 264:cfg.ROW],
                        zero_wide[:, 0:BT * (cfg.ROW - 264)].rearrange(
                            "p (s d) -> p s d", s=BT),
                        mybir.ActivationFunctionType.Copy)
                    xcf = xc[:].bitcast(F32)  # [128, BT, 192]
                    arow = pool.tile([128, BT, 4], F32,
                                     name=f"ar{id(xcat)}_{bi}", tag="ar")
                    for s in range(BT):
                        ps = psA.tile([128, 264], F32, name=f"dps{bi}_{s}",
                                      tag="dps")
                        nc.tensor.matmul(
                            ps[0:pc, :], xTs[:, s, :],
                            wa_sb[:], start=True, stop=True)
                        nc.scalar.activation(
                            xc[0:pc, s, 0:256], ps[0:pc, 0:256],
                            mybir.ActivationFunctionType.Copy)
                        nc.vector.tensor_copy(xcf[0:pc, s, 128:132],
                                              ps[0:pc, 256:260])
                        nc.vector.tensor_copy(arow[0:pc, s, :],
                                              ps[0:pc, 260:264])
                    nc.sync.dma_start(
                        out=xcat[nb:nb + bsz, :].rearrange(
                            "(p s) d -> p s d", s=BT),
                        in_=xc[0:pc, 0:BT, :])
                    nc.sync.dma_start(
                        out=aldf[nb:nb + bsz, :].rearrange(
                            "(p s) d -> p s d", s=BT),
                        in_=arow[0:pc, 0:BT, :])
                    nb += bsz
                    bi += 1

            def ald_gather(aldf, aldl):
                asb = pool.tile([128, n_ald // 128, 64], F32, tag="asb")
                nc.gpsimd.dma_gather(
                    asb[:],
                    aldf[:, :].rearrange("(g k) d -> g (k d)", k=16),
                    aldg_sb[:], n_ald, n_ald, 64, single_packet=False,
                    queue_num=q_next())
                nc.sync.dma_start(
                    out=aldl[:, :].rearrange("(c p j) d -> p c (j d)",
                                             p=128, j=16),
                    in_=asb[:])

            def sweep_a(xcat, n_rows, aldl, layer, t):
                """Per-tile stage A: loads, gathers, sel, ald, alpha, gw."""
                sfx = f"_{layer}_{t}"
                q = t % 4
                idx_t = ldpool.tile([128, C * 8], I16, name="ix" + sfx,
                                    tag="ix")
                nc.sync.dma_start(out=idx_t[:], in_=gidx_d[t, :, :])
                selt_t = stpool.tile([128, C * 128], BF16, name="sT" + sfx,
                                     tag="sT")
                nc.sync.dma_start(out=selt_t[:], in_=selt_d[t, :, :])
                ald32 = ldpool.tile([128, 4], F32, name="at" + sfx, tag="at")
                nc.sync.dma_start(out=ald32[:],
                                  in_=aldl[t * 128:(t + 1) * 128, :])
                ald16 = ldpool.tile([128, 4], BF16, name="a6" + sfx, tag="a6")
                nc.vector.tensor_copy(ald16[:], ald32[:])

                r_lo, r_hi = regs[q]
                nc.gpsimd.reg_load(r_lo, cnt_sb[0:1, 2 * t:2 * t + 1])
                nc.gpsimd.reg_load(r_hi, cnt_sb[0:1, 2 * t + 1:2 * t + 2])
                G = gpool.tile([128, C, cfg.ROW], BF16, name="G" + sfx,
                               tag="G")
                # zero first: -1-padded gather slots are skipped, and stale
                # bytes would poison agg via 0 x NaN in the scatter matmul
                nc.scalar.activation(
                    G[:].rearrange("p c f -> p (c f)"), zero_wide[:],
                    mybir.ActivationFunctionType.Copy)
                nc.gpsimd.dma_gather(
                    G[:, 0:C_lo, :], xcat[0:cfg.SPLIT, :],
                    idx_t[:, 0:C_lo * 8], C_lo * 128, r_lo,
                    cfg.ROW, single_packet=SINGLE_PACKET,
                    queue_num=q_next())
                nc.gpsimd.dma_gather(
                    G[:, C_lo:C, :], xcat[cfg.SPLIT:n_rows, :],
                    idx_t[:, C_lo * 8:], C_hi * 128, r_hi,
                    cfg.ROW, single_packet=SINGLE_PACKET,
                    queue_num=q_next())
                Gf = G[:].bitcast(F32)  # [128, C, 192]

                # one-hot (edge-major) for the agg matmul stationaries
                sd3 = sdpool.tile([128, C * 128], BF16, name="sd" + sfx,
                                  tag="sd")
                nc.sync.dma_start(out=sd3[:], in_=seld_d[t, :, :])
                sd = sd3[:].rearrange("p (c j) -> p c j", j=128)

                # per-edge ald via host selT: ald_e = selT_c^T @ ald16
                ald_ps = psAld.tile([128, C * 4], F32, name="alp" + sfx,
                                    tag="alp")
                sT3 = selt_t[:].rearrange("p (c j) -> p c j", j=128)
                for c in range(C):
                    nc.tensor.matmul(
                        ald_ps[:, c * 4:(c + 1) * 4], sT3[:, c, :], ald16[:],
                        start=True, stop=True, skip_group_check=True)

                alpha = wpool.tile([128, C, 4], F32, name="alf" + sfx,
                                   tag="alf")
                nc.vector.tensor_tensor(
                    out=alpha[:], in0=Gf[:, :, 128:132],
                    in1=ald_ps[:].rearrange("p (c f) -> p c f", f=4),
                    op=ADD)
                alr = wpool.tile([128, C, 4], F32, name="alr" + sfx,
                                 tag="alr")
                nc.vector.scalar_tensor_tensor(
                    out=alr[:], in0=alpha[:], scalar=NEG_SLOPE, in1=alpha[:],
                    op0=MULT, op1=MAXOP)
                nc.vector.tensor_scalar(alr[:], alr[:], ALPHA_CAP, None,
                                        MINOP)
                wb = wpool.tile([128, C * 4], BF16, name="wb" + sfx, tag="wb")
                nc.scalar.activation(
                    wb[:].rearrange("p (c f) -> p c f", f=4), alr[:],
                    mybir.ActivationFunctionType.Exp)

                gw = gwpool.tile([128, C, 260], BF16, name="gw" + sfx,
                                 tag="gw")
                nc.vector.tensor_tensor(
                    out=gw[:, :, 0:256].rearrange("p c (h f) -> p c h f",
                                                  f=64),
                    in0=G[:, :, 0:256].rearrange("p c (h f) -> p c h f",
                                                 f=64),
                    in1=wb[:].rearrange("p (c h) -> p c h", h=4).unsqueeze(3)
                        .broadcast_to([128, C, 4, 64]),
                    op=MULT)
                nc.vector.tensor_copy(
                    gw[:, :, 256:260],
                    wb[:].rearrange("p (c h) -> p c h", h=4))
                agg = psAgg.tile([128, 260], F32, name="agg" + sfx, tag="agg")
                if DEBUG_DUMP and layer == 1 and t == 0:
                    for dt_, src_ap in [
                            (dbg_sd, sd3[:]),
                            (dbg_alpha,
                             alpha[:].rearrange("p c f -> p (c f)")),
                            (dbg_wb, wb[:]),
                            (dbg_gw, gw[:].rearrange("p c f -> p (c f)")),
                            (dbg_aldps, ald_ps[:]),
                            (dbg_g, G[:].rearrange("p c f -> p (c f)"))]:
                        tmpd = wpool.tile(list(dt_.shape), F32,
                                          name=f"dbg{dt_.name}", tag="dbg",
                                          bufs=1)
                        nc.vector.tensor_copy(tmpd[:], src_ap)
                        nc.sync.dma_start(out=dt_[:, :], in_=tmpd[:])
                return dict(sd=sd, gw=gw, agg=agg, t=t)

            def sweep_b(st_dict, layer):
                """Per-tile stage B: agg matmuls, normalize, ELU/store."""
                t = st_dict["t"]
                sfx = f"_{layer}_{t}"
                sd, gw, agg = st_dict["sd"], st_dict["gw"], st_dict["agg"]
                for c in range(C):
                    nc.tensor.matmul(
                        agg[:, 0:260], sd[:, c, :], gw[:, c, :],
                        start=(c == 0), stop=(c == C - 1),
                        skip_group_check=True)
                den = bpool.tile([128, 4], F32, name="dn" + sfx, tag="dn")
                nc.vector.tensor_scalar(den[:], agg[:, 256:260], 4.0, 4e-16,
                                        MULT, MAXOP)
                rec = bpool.tile([128, 4], F32, name="rc" + sfx, tag="rc")
                nc.vector.reciprocal(rec[:], den[:])
                tmp = bpool.tile([128, 4, 64], F32, name="tm" + sfx,
                                 tag="tm")
                nc.vector.tensor_tensor(
                    out=tmp[:],
                    in0=agg[:, 0:256].rearrange("p (h f) -> p h f", f=64),
                    in1=rec[:].unsqueeze(2).broadcast_to([128, 4, 64]),
                    op=MULT)
                s0 = bpool.tile([128, 64], F32, name="s0" + sfx, tag="s0")
                s1 = bpool.tile([128, 64], F32, name="s1" + sfx, tag="s1")
                nc.vector.tensor_tensor(out=s0[:], in0=tmp[:, 0, :],
                                        in1=tmp[:, 1, :], op=ADD)
                nc.vector.tensor_tensor(out=s1[:], in0=tmp[:, 2, :],
                                        in1=tmp[:, 3, :], op=ADD)
                nc.vector.tensor_tensor(out=s0[:], in0=s0[:], in1=s1[:],
                                        op=ADD)
                if layer == 1:
                    # ELU(s) = max(s,0) + exp(min(s,0)) - 1
                    ng = bpool.tile([128, 64], F32, name="ng" + sfx,
                                    tag="ng")
                    nc.vector.tensor_scalar(ng[:], s0[:], 0.0, None, MINOP)
                    ex = bpool.tile([128, 64], F32, name="ex" + sfx,
                                    tag="ex")
                    nc.scalar.activation(
                        ex[:], ng[:], mybir.ActivationFunctionType.Exp)
                    nc.vector.tensor_scalar(s0[:], s0[:], 0.0, None, MAXOP)
                    nc.vector.tensor_tensor(out=s0[:], in0=s0[:], in1=ex[:],
                                            op=ADD)
                    nc.vector.tensor_scalar(s0[:], s0[:], 1.0, None, SUB)
                    nc.sync.dma_start(
                        out=h_loc[t * 128:(t + 1) * 128, 0:64], in_=s0[:])
                else:
                    nc.sync.dma_start(
                        out=out_d[t * 128:(t + 1) * 128, :], in_=s0[:])

            def edge_sweep(xcat, n_rows, aldl, layer):
                prev = None
                for t in range(cfg.T):
                    cur = sweep_a(xcat, n_rows, aldl, layer, t)
                    if prev is not None:
                        sweep_b(prev, layer)
                    prev = cur
                sweep_b(prev, layer)

            # ============ layer 1 ============
            dense_phase(xs16, NP1, cfg.IN, wa1_sb, xcat1, aldf1)
            ald_gather(aldf1, aldl1)
            edge_sweep(xcat1, NP1, aldl1, 1)

            # ============ exchange ============
            nc.gpsimd.collective_compute(
                "AllGather", mybir.AluOpType.bypass,
                replica_groups=[list(range(cfg.NCORES))],
                ins=[h_loc.opt()], outs=[h_full.opt()])
            align_pool_q0(xcat1)
            pctr[0] += 1  # Pool dma_start below rides queue 0
            nc.gpsimd.dma_start(
                out=h16[:, :].flatten(),
                in_=h_full[:, :].flatten())

            # ============ layer 2 ============
            dense_phase(h16, NP2, cfg.H, wa2_sb, xcat2, aldf2)
            ald_gather(aldf2, aldl2)
            edge_sweep(xcat2, NP2, aldl2, 2)

    nc.compile()
    return nc


def _run(cfg, inputs, run_fn):
    prep = host_prep(cfg, inputs["edge_index"])
    wa1 = _weights_cat(np.asarray(inputs["W1"], np.float32),
                       np.asarray(inputs["a_src1"], np.float32),
                       np.asarray(inputs["a_dst1"], np.float32),
                       cfg.HEADS, cfg.H)
    wa2 = _weights_cat(np.asarray(inputs["W2"], np.float32),
                       np.asarray(inputs["a_src2"], np.float32),
                       np.asarray(inputs["a_dst2"], np.float32),
                       cfg.HEADS, cfg.OUT)
    x = np.ascontiguousarray(np.asarray(inputs["x"], np.float32))

    nc = build_kernel(cfg, prep["C_lo"], prep["C_hi"], prep["n_ald"])
    in_maps = []
    for c in range(cfg.NCORES):
        in_maps.append({
            "x": x, "wa1": wa1, "wa2": wa2,
            "gidx": prep["gidx"][c],
            "selt": prep["selt"][c], "seld": prep["seld"][c],
            "cnt": prep["cnt"][c], "aldg": prep["aldg"][c],
        })
    results = run_fn(nc, in_maps)
    out = np.concatenate([results[c]["out_slice"]
                          for c in range(cfg.NCORES)], axis=0)
    return out[:cfg.N]


def kernel(**inputs) -> np.ndarray:
    cfg = FULL

    def run_fn(nc, in_maps):
        res = run_bass_kernel_spmd(
            nc, in_maps, core_ids=list(range(cfg.NCORES)),
            tmpdir=os.environ.get("GAT_TMPDIR") or None,
            trace=os.environ.get("GAT_TRACE", "0") == "1")
        if res.exec_time_ns is not None:
            print(f"HW exec time: {res.exec_time_ns} ns")
        return res.results

    return _run(cfg, inputs, run_fn)
